# revision 20
# baseline (speedup 1.0000x reference)
"""CapsuleConv2d (3x3, stride 1, pad 1, L_in=4, L_out=8, 3 routing iters) on 8 trn2 cores.

Sharding: data-parallel over (N=4 images) x (2 half-images of 28 rows) = 8 shards.
Each core computes priors via PE matmuls (block-diag weight over capsule groups),
then dynamic routing with positions on the partition axis:
  - PE: priors u (and the uniform-probs first vote s0, folded into the matmul)
  - DVE: elementwise products, segmented reductions, softmax pieces, squash
    (GPSIMD was tried for the big products and measured ~3x slower on these
    strided/broadcast access patterns - software address generation)
  - ACT: PSUM->SBUF copies, exp, sqrt

Engine ISA instructions carry at most one semaphore wait in hardware; building
on bacc.Bacc (not bass.Bass) runs generate_event_semaphores at finalize, which
legalizes the multi-waits Tile emits for cross-engine dependencies.

Per-position free-dim layout for priors u[c,m,k,g]:  idx = c*576 + m*72 + k*8 + g
  c = out-capsule (8), m = out-length (8), k = kernel offset (9), g = in-capsule (8)
"""

import numpy as np

import concourse.bass as bass
import concourse.mybir as mybir
import concourse.tile as tile
from concourse import bacc

FP32 = mybir.dt.float32
AF = mybir.ActivationFunctionType
MULT = mybir.AluOpType.mult

KK, GI, GO, LI, LO = 9, 8, 8, 4, 8
HO = WO = 56
ROWS = 28            # output rows per core
SH, SW = ROWS + 2, WO + 2   # 30 x 58 padded input slice
POS = ROWS * WO      # 1568 positions per core
TP = 114             # 2 output rows + 2 junk pad positions per tile
NT = 14              # tiles per core (2 rows each)
CM = GO * LO         # 64
CKG = GO * KK * GI   # 576 (c,k,g)
UF = GO * LO * KK * GI  # 4608 (c,m,k,g)

# free-dim strides in u
SC, SM, SK, SG = 576, 72, 8, 1
INF = SH * SW + KK * 512 + KK * CM  # fused input columns: x | wmm | wsum


def _v(a, dims):
    """Re-view an AP (taken at a tile's origin) with explicit free [step,count] dims."""
    return bass.AP(a.tensor, a.offset, [list(a.ap[0])] + [list(d) for d in dims])


def build_program(nt=NT):
    nc = bacc.Bacc()
    # single fused input (one DMA, one semaphore -> LDWEIGHTS can encode the wait)
    inp = nc.dram_tensor("inp", [32, INF], FP32, kind="ExternalInput")
    out = nc.dram_tensor("out", [NT * TP, CM], FP32, kind="ExternalOutput")

    with tile.TileContext(nc) as tc:
        with (
            tc.tile_pool(name="singles", bufs=1) as singles,
            tc.tile_pool(name="upool", bufs=2) as upool,
            tc.tile_pool(name="ttpool", bufs=4) as ttpool,
            tc.tile_pool(name="mid", bufs=4) as mid,
            tc.tile_pool(name="tiny", bufs=3) as tiny,
            tc.tile_pool(name="vout", bufs=3) as vout,
            tc.tile_pool(name="pu", bufs=3, space="PSUM") as pupool,
            tc.tile_pool(name="ps0", bufs=2, space="PSUM") as ps0pool,
        ):
            inp_sb = singles.tile([32, INF], FP32)
            nc.sync.dma_start(out=inp_sb[:], in_=inp[:])
            xs_flat = inp_sb[:, :SH * SW]
            wmm_sb = inp_sb[:, SH * SW:SH * SW + KK * 512].rearrange(
                "p (k n) -> p k n", k=KK)
            wsum_sb = inp_sb[:, SH * SW + KK * 512:].rearrange(
                "p (k n) -> p k n", k=KK)

            for t in range(nt):
                h0 = 2 * t
                # ---- priors: u[pos; c,m,k,g] and s0[pos; c,m] on PE ----
                u = upool.tile([TP, UF], FP32)
                ps0 = ps0pool.tile([TP, CM], FP32)
                for k in range(KK):
                    di, dj = k // 3, k % 3
                    # flat 114-run covering 2 rows of 56 (+2 junk at 56,57):
                    # LDWEIGHTS needs a single-free-dim AP
                    o = (h0 + di) * SW + dj
                    lhsT = xs_flat[:, o:o + TP]  # [32, 114]
                    pu = pupool.tile([TP, 512], FP32)
                    nc.tensor.matmul(pu[:], lhsT, wmm_sb[:, k, :], start=True, stop=True)
                    nc.tensor.matmul(ps0[:], lhsT, wsum_sb[:, k, :],
                                     start=(k == 0), stop=(k == KK - 1))
                    # psum (c,m,g) -> sbuf u[:, c,m,k=k,g]  (strided write, ACT)
                    u4 = _v(u[:], [[SC, GO], [SM, LO], [SK, KK], [SG, GI]])
                    nc.scalar.copy(out=u4[:, :, :, k, :], in_=pu[:])

                # ---- routing ----
                def squash(s_ap, vdst):
                    sq = tiny.tile([TP, CM], FP32, tag="sq")
                    nc.vector.tensor_mul(sq[:], s_ap, s_ap)
                    n2 = tiny.tile([TP, GO], FP32, tag="n2")
                    nc.vector.reduce_sum(n2[:], _v(sq[:], [[LO, GO], [1, LO]]),
                                         axis=mybir.AxisListType.X)
                    rt = tiny.tile([TP, GO], FP32, tag="rt")
                    nc.scalar.activation(rt[:], n2[:], AF.Sqrt)
                    n2p1 = tiny.tile([TP, GO], FP32, tag="n2p1")
                    nc.scalar.add(n2p1[:], n2[:], 1.0)
                    inv = tiny.tile([TP, GO], FP32, tag="inv")
                    nc.vector.reciprocal(inv[:], n2p1[:])
                    phi = tiny.tile([TP, GO], FP32, tag="phi")
                    nc.vector.tensor_mul(phi[:], rt[:], inv[:])
                    # v = s * phi (phi broadcast over m)
                    return nc.vector.tensor_tensor(
                        _v(vdst[:], [[LO, GO], [1, LO]]),
                        bass.AP(s_ap.tensor, s_ap.offset,
                                [list(s_ap.ap[0]), [LO, GO], [1, LO]]),
                        _v(phi[:], [[1, GO], [0, LO]]),
                        op=MULT)

                s0 = tiny.tile([TP, CM], FP32, tag="s0")
                nc.scalar.copy(out=s0[:], in_=ps0[:])
                v = vout.tile([TP, CM], FP32, tag="v")
                squash(s0[:], v)

                b_prev = None
                for r in (1, 2):
                    # tt = u * v  (v[c,m] broadcast over k,g)
                    tt = ttpool.tile([TP, UF], FP32, tag="tt")
                    nc.vector.tensor_tensor(
                        _v(tt[:], [[SC, GO], [SM, LO], [1, KK * GI]]),
                        _v(u[:], [[SC, GO], [SM, LO], [1, KK * GI]]),
                        _v(v[:], [[LO, GO], [1, LO], [0, KK * GI]]),
                        op=MULT)
                    # b = sum_m tt  -> [pos; c,k,g]
                    b = mid.tile([TP, CKG], FP32, tag="b")
                    nc.vector.reduce_sum(
                        b[:], _v(tt[:], [[SC, GO], [SK, KK], [SG, GI], [SM, LO]]),
                        axis=mybir.AxisListType.X)
                    if b_prev is not None:
                        nc.vector.tensor_add(b[:], b[:], b_prev[:])
                    b_prev = b
                    # softmax over k (segments of the c,k,g layout)
                    e = mid.tile([TP, CKG], FP32, tag="e")
                    nc.scalar.activation(e[:], b[:], AF.Exp)
                    ssum = tiny.tile([TP, CM], FP32, tag="ssum")
                    nc.vector.reduce_sum(
                        ssum[:], _v(e[:], [[KK * GI, GO], [SG, GI], [SK, KK]]),
                        axis=mybir.AxisListType.X)
                    invs = tiny.tile([TP, CM], FP32, tag="invs")
                    nc.vector.reciprocal(invs[:], ssum[:])
                    p = mid.tile([TP, CKG], FP32, tag="p")
                    nc.vector.tensor_tensor(
                        _v(p[:], [[KK * GI, GO], [SK, KK], [SG, GI]]),
                        _v(e[:], [[KK * GI, GO], [SK, KK], [SG, GI]]),
                        _v(invs[:], [[GI, GO], [0, KK], [1, GI]]),
                        op=MULT)
                    # tt2 = p * u ; s = sum_{k,g} tt2
                    tt2 = ttpool.tile([TP, UF], FP32, tag="tt")
                    nc.vector.tensor_tensor(
                        _v(tt2[:], [[SC, GO], [SM, LO], [SK, KK], [SG, GI]]),
                        _v(u[:], [[SC, GO], [SM, LO], [SK, KK], [SG, GI]]),
                        _v(p[:], [[KK * GI, GO], [0, LO], [SK, KK], [SG, GI]]),
                        op=MULT)
                    s = tiny.tile([TP, CM], FP32, tag="s")
                    nc.vector.reduce_sum(
                        s[:], _v(tt2[:], [[SC, GO], [SM, LO], [SK, KK], [SG, GI]]),
                        axis=mybir.AxisListType.XY)
                    v = vout.tile([TP, CM], FP32, tag="v")
                    squash(s[:], v)

                nc.sync.dma_start(out=out[t * TP:(t + 1) * TP, :], in_=v[:])
    return nc


_PROG = None


def _get_program():
    global _PROG
    if _PROG is None:
        _PROG = build_program()
        _PROG.finalize()
    return _PROG


def _prep_weights(w):
    wr = np.ascontiguousarray(w.reshape(GO, GI, KK, LI, LO), np.float32)
    # wmm[(g,l), k, (c,m,g2)] = wr[c,g,k,l,m] iff g2 == g
    wmm6 = np.zeros((GI, LI, KK, GO, LO, GI), np.float32)
    for g in range(GI):
        # wr[:, g] is (c,k,l,m) -> (l,k,c,m)
        wmm6[g, :, :, :, :, g] = np.transpose(wr[:, g], (2, 1, 0, 3))
    wmm = wmm6.reshape(32, KK * 512)
    # wsum[(g,l), k, (c,m)] = wr[c,g,k,l,m]/9
    wsum = (np.transpose(wr, (1, 3, 2, 0, 4)) / 9.0).reshape(32, KK * CM)
    return np.ascontiguousarray(wmm), np.ascontiguousarray(wsum.astype(np.float32))


def make_in_maps(x, weight):
    x = np.asarray(x, np.float32)
    xp = np.pad(x, ((0, 0), (0, 0), (1, 1), (1, 1)))
    wmm, wsum = _prep_weights(np.asarray(weight, np.float32))
    in_maps = []
    for core in range(8):
        n, h0 = core // 2, (core % 2) * ROWS
        xsl = np.ascontiguousarray(xp[n, :, h0:h0 + SH, :].reshape(32, SH * SW))
        in_maps.append({"inp": np.ascontiguousarray(
            np.concatenate([xsl, wmm, wsum], axis=1))})
    return in_maps


def assemble(results):
    out = np.zeros((4, GO * LO, HO, WO), np.float32)
    for core in range(8):
        n, h0 = core // 2, (core % 2) * ROWS
        o = np.asarray(results[core]["out"]).reshape(NT, TP, CM)
        # TP=114 run: [0:56] = row 0, [58:114] = row 1, 56/57 junk
        o = np.stack([o[:, :WO], o[:, SW:SW + WO]], axis=1)  # (NT, 2, 56, CM)
        o = o.reshape(ROWS, WO, CM)
        out[n, :, h0:h0 + ROWS, :] = np.transpose(o, (2, 0, 1))
    return out


def kernel(x, weight):
    from concourse.bass_utils import run_bass_kernel_spmd
    nc = _get_program()
    res = run_bass_kernel_spmd(nc, make_in_maps(x, weight), list(range(8)))
    return assemble(res.results)
